# revision 10
# baseline (speedup 1.0000x reference)
"""DeepSeek MLA attention (B=1, T=2048, D=2048, H=16) on 8 trn2 NeuronCores.

Tensor-parallel over heads: 2 heads per core. wq_b / wkv_b output dims and
wo input dim are head-sharded; x, wq_a, wkv_a replicated. Each core produces
a partial [T, D] o_proj output; the host sums the 8 partials (the all-reduce).

All on-device activations live in transposed [feature-partition, T-free]
layouts so every matmul contraction sits on the partition axis. Matmul inputs
are bf16 (fp32 accumulation in PSUM). RMS-norm is a per-token column scale
(norm weights are folded into wq_b/wkv_b rows on the host); RoPE is applied
via a PE half-swap permutation matmul plus sign-folded sin/cos tables.
"""

import sys

if "/opt/trn_rl_repo" not in sys.path:
    sys.path.insert(0, "/opt/trn_rl_repo")

from contextlib import ExitStack

import ml_dtypes
import numpy as np

import concourse.bass as bass
import concourse.tile as tile
from concourse import bacc, mybir
from concourse.bass import ts
from concourse.bass_utils import run_bass_kernel_spmd
from concourse.masks import make_identity

BF16 = ml_dtypes.bfloat16
F32 = mybir.dt.float32
BF = mybir.dt.bfloat16

T, D = 2048, 2048
H, DK = 16, 128
ROPE, NOPE, VD = 64, 64, 128
HALF = ROPE // 2
KVR, QR = 512, 768
KVW = KVR + 128  # wkv_a padded: [0:512]=c, [512:576]=0, [576:640]=k_pe
SCALE = DK ** -0.5
EPS = float(np.finfo(np.float32).eps)
N_CORES = 8
HL = H // N_CORES  # heads per core

TCH = 512                 # T-chunk for projection phase
QC = 512                  # q-chunk for attention phase
P = 128


def _emit(nc, tc, ctx, t_len, aps):
    (xT, wqa, wkva, wqb, wkvbn, wkvbv, wo_l, cosA, sinA2, swapM, o_out) = aps
    NT = t_len // TCH          # projection chunks
    NTT = t_len // P           # token tiles
    QCL = min(QC, t_len)       # attention q-chunk size
    NQC = t_len // QCL         # attention q chunks
    TPC = TCH // P             # token tiles per chunk
    QRT = QR // P   # 6
    CRT = KVR // P  # 4
    DTN = D // P    # 16

    consts = ctx.enter_context(tc.tile_pool(name="consts", bufs=1))
    persist = ctx.enter_context(tc.tile_pool(name="persist", bufs=1))

    # constants
    ones128 = consts.tile([P, P], BF)
    nc.gpsimd.memset(ones128, 1.0)
    ident = consts.tile([P, P], F32)
    make_identity(nc, ident)
    eps_ap = consts.tile([P, 1], F32)
    nc.vector.memset(eps_ap, EPS)

    # persistent activations
    qsT = persist.tile([P, HL, t_len], BF)       # [DK, h, T] (nope | roped pe)
    ksT = persist.tile([P, HL, t_len], BF)
    v_sb = persist.tile([P, HL, NTT, VD], BF)    # [token-in-tile, h, ttile, VD]
    outT = persist.tile([P, HL, t_len], BF)      # [VD, h, T]
    s_cT = persist.tile([P, NTT], F32)           # per-token kv rms scale, col layout

    # ================= phase 1: projections =================
    with tc.tile_pool(name="w1", bufs=1) as wpool, \
         tc.tile_pool(name="xchunk", bufs=2) as xpool, \
         tc.tile_pool(name="raw", bufs=2) as rawpool, \
         tc.tile_pool(name="sq", bufs=3) as sqpool, \
         tc.tile_pool(name="scales", bufs=2) as scpool, \
         tc.tile_pool(name="ropetmp", bufs=2) as tmpool, \
         tc.tile_pool(name="ps_proj", bufs=3, space="PSUM") as ps_proj, \
         tc.tile_pool(name="ps_sum", bufs=1, space="PSUM") as ps_sum, \
         tc.tile_pool(name="ps_small", bufs=3, space="PSUM") as ps_small:

        # interleave chunk-0 x tiles with the projection weights so the
        # first matmul group can start after ~3 small DMAs
        wqa_sb = wpool.tile([P, DTN, QR], BF)
        wkva_sb = wpool.tile([P, DTN, KVW], BF)
        xc0 = xpool.tile([P, DTN, TCH], BF, tag="xc")
        for dt in range(DTN):
            nc.sync.dma_start(out=xc0[:, dt, :], in_=xT[ts(dt, P), 0:TCH])
            nc.sync.dma_start(out=wqa_sb[:, dt, :], in_=wqa[ts(dt, P), :])
            nc.sync.dma_start(out=wkva_sb[:, dt, :], in_=wkva[ts(dt, P), :])
        wqb_sb = wpool.tile([P, QRT, HL, DK], BF)
        nc.sync.dma_start(out=wqb_sb, in_=wqb.rearrange("(r p) h d -> p r h d", p=P))
        wkvbn_sb = wpool.tile([P, CRT, HL, NOPE], BF)
        nc.sync.dma_start(out=wkvbn_sb, in_=wkvbn.rearrange("(r p) h d -> p r h d", p=P))
        wkvbv_sb = wpool.tile([P, CRT, HL, VD], BF)
        nc.sync.dma_start(out=wkvbv_sb, in_=wkvbv.rearrange("(r p) h d -> p r h d", p=P))
        cosA_sb = wpool.tile([P, t_len], BF)
        nc.sync.dma_start(out=cosA_sb, in_=cosA)
        sinA2_sb = wpool.tile([P, t_len], BF)
        nc.sync.dma_start(out=sinA2_sb, in_=sinA2)
        swapM_sb = wpool.tile([P, P], BF)
        nc.sync.dma_start(out=swapM_sb, in_=swapM)

        for c in range(NT):
            tsl = slice(c * TCH, (c + 1) * TCH)
            if c == 0:
                xc = xc0
            else:
                xc = xpool.tile([P, DTN, TCH], BF, tag="xc")
                nc.sync.dma_start(
                    out=xc, in_=xT[:, tsl].rearrange("(dt p) t -> p dt t", p=P)
                )

            qa_r = rawpool.tile([P, QRT, TCH], BF, tag="qa_raw")
            c_r = rawpool.tile([P, CRT, TCH], BF, tag="c_raw")
            kpe_r = rawpool.tile([P, TCH], BF, tag="kpe_raw")  # rows 64:128 = k_pe

            sq_q_ps = ps_sum.tile([P, TCH], F32, tag="sq_q")
            sq_c_ps = ps_sum.tile([P, TCH], F32, tag="sq_c")

            scale_q = scpool.tile([P, TCH], F32, tag="scale_q")
            scale_c = scpool.tile([P, TCH], F32, tag="scale_c")

            # x @ [wq_a | wkv_a] (transposed). The ones-matmul (sum of
            # squares) for group r is emitted after group r+1's matmuls so
            # the PE never waits on the ACT Square latency.
            pending = None

            def flush_pending():
                nonlocal pending
                if pending is not None:
                    tgt, sqt, st, sp = pending
                    nc.tensor.matmul(tgt, ones128, sqt, start=st, stop=sp)
                    pending = None

            for r in range(QRT + CRT + 1):
                ps = ps_proj.tile([P, TCH], F32, tag="proj")
                if r < QRT:
                    w, col = wqa_sb, ts(r, P)
                elif r < QRT + CRT:
                    w, col = wkva_sb, ts(r - QRT, P)
                else:
                    w, col = wkva_sb, ts(CRT, P)  # padded pe block -> rows 64:128
                for dt in range(DTN):
                    nc.tensor.matmul(
                        ps, w[:, dt, col], xc[:, dt, :],
                        start=(dt == 0), stop=(dt == DTN - 1),
                    )
                flush_pending()
                if r < QRT:
                    nc.vector.tensor_copy(out=qa_r[:, r, :], in_=ps)
                    sq = sqpool.tile([P, TCH], BF, tag="sq")
                    nc.scalar.activation(out=sq, in_=ps,
                                         func=mybir.ActivationFunctionType.Square)
                    pending = (sq_q_ps, sq, r == 0, r == QRT - 1)
                elif r < QRT + CRT:
                    rc = r - QRT
                    nc.vector.tensor_copy(out=c_r[:, rc, :], in_=ps)
                    sq = sqpool.tile([P, TCH], BF, tag="sq")
                    nc.scalar.activation(out=sq, in_=ps,
                                         func=mybir.ActivationFunctionType.Square)
                    pending = (sq_c_ps, sq, rc == 0, rc == CRT - 1)
                else:
                    nc.vector.tensor_copy(out=kpe_r, in_=ps)
                # rms scale chains, emitted as soon as their sumsq closes
                if r == QRT:  # q sumsq flushed above
                    tmp_q = scpool.tile([P, TCH], F32, tag="scale_tmp")
                    nc.scalar.activation(out=tmp_q, in_=sq_q_ps,
                                         func=mybir.ActivationFunctionType.Sqrt,
                                         scale=1.0 / QR, bias=eps_ap)
                    nc.vector.reciprocal(out=scale_q, in_=tmp_q)
            flush_pending()  # c sumsq (last group has no successor)
            tmp_c = scpool.tile([P, TCH], F32, tag="scale_tmp")
            nc.scalar.activation(out=tmp_c, in_=sq_c_ps,
                                 func=mybir.ActivationFunctionType.Sqrt,
                                 scale=1.0 / KVR, bias=eps_ap)
            nc.vector.reciprocal(out=scale_c, in_=tmp_c)

            # rope tables pre-scaled by the q rms scale (rows 64:128 only)
            cos_s = scpool.tile([P, TCH], BF, tag="cos_s")
            sin_s = scpool.tile([P, TCH], BF, tag="sin_s")
            nc.vector.tensor_mul(cos_s[64:128, :], cosA_sb[64:128, tsl],
                                 scale_q[64:128, :])
            nc.vector.tensor_mul(sin_s[64:128, :], sinA2_sb[64:128, tsl],
                                 scale_q[64:128, :])

            # ---- b-projections. PE order: qsT/kn matmul groups first, then
            # the scale_c transposes, v groups, and finally the rope swaps —
            # so every PE instruction's inputs are ready when it issues.
            q_ps = []
            for h in range(HL):
                ps = ps_proj.tile([P, TCH], F32, tag="proj")
                for r in range(QRT):
                    nc.tensor.matmul(ps, wqb_sb[:, r, h, :], qa_r[:, r, :],
                                     start=(r == 0), stop=(r == QRT - 1))
                nc.vector.tensor_mul(qsT[0:64, h, tsl], ps[0:64, :],
                                     scale_q[0:64, :])
                qpe_raw = tmpool.tile([P, TCH], BF, tag=f"qpe_raw{h}")
                nc.vector.tensor_copy(out=qpe_raw, in_=ps)
                q_ps.append(qpe_raw)

            kn_ps_l = []
            for h in range(HL):
                kn_ps = ps_small.tile([64, TCH], F32, tag="small")
                for r in range(CRT):
                    nc.tensor.matmul(kn_ps, wkvbn_sb[:, r, h, :], c_r[:, r, :],
                                     start=(r == 0), stop=(r == CRT - 1))
                nc.vector.tensor_mul(ksT[0:64, h, tsl], kn_ps, scale_c[0:64, :])
                kn_ps_l.append(kn_ps)

            # column-layout copy of scale_c for per-partition v scaling
            for j in range(TPC):
                tr_ps = ps_small.tile([P, P], F32, tag="small")
                nc.tensor.transpose(tr_ps, scale_c[:, ts(j, P)], ident)
                nc.vector.tensor_copy(out=s_cT[:, c * TPC + j: c * TPC + j + 1],
                                      in_=tr_ps[:, 0:1])

            for h in range(HL):
                for j in range(TPC):
                    tt = c * TPC + j
                    v_ps = ps_small.tile([P, VD], F32, tag="small")
                    for r in range(CRT):
                        nc.tensor.matmul(v_ps, c_r[:, r, ts(j, P)],
                                         wkvbv_sb[:, r, h, :],
                                         start=(r == 0), stop=(r == CRT - 1))
                    nc.vector.tensor_scalar_mul(v_sb[:, h, tt, :], v_ps,
                                                s_cT[:, tt:tt + 1])

            # rope swaps (inputs were evicted long before these issue)
            for h in range(HL):
                sw_ps = ps_proj.tile([P, TCH], F32, tag="proj")
                nc.tensor.matmul(sw_ps, swapM_sb, q_ps[h], start=True, stop=True)
                m1 = tmpool.tile([P, TCH], BF, tag="rope_m1")
                m2 = tmpool.tile([P, TCH], BF, tag="rope_m2")
                nc.vector.tensor_mul(m1[64:128, :], q_ps[h][64:128, :],
                                     cos_s[64:128, :])
                nc.vector.tensor_mul(m2[64:128, :], sw_ps[64:128, :],
                                     sin_s[64:128, :])
                nc.vector.tensor_add(qsT[64:128, h, tsl], m1[64:128, :],
                                     m2[64:128, :])

            ksw_ps = ps_proj.tile([P, TCH], F32, tag="proj")
            nc.tensor.matmul(ksw_ps, swapM_sb, kpe_r, start=True, stop=True)
            km1 = tmpool.tile([P, TCH], BF, tag="rope_m1")
            km2 = tmpool.tile([P, TCH], BF, tag="rope_m2")
            nc.vector.tensor_mul(km1[64:128, :], kpe_r[64:128, :],
                                 cosA_sb[64:128, tsl])
            nc.vector.tensor_mul(km2[64:128, :], ksw_ps[64:128, :],
                                 sinA2_sb[64:128, tsl])
            nc.vector.tensor_add(ksT[64:128, 0, tsl], km1[64:128, :],
                                 km2[64:128, :])
            nc.vector.tensor_add(ksT[64:128, 1, tsl], km1[64:128, :],
                                 km2[64:128, :])

    # ================= phase 2+3: attention, then o_proj =================
    with tc.tile_pool(name="w2", bufs=1) as wpool2, \
         tc.tile_pool(name="probs", bufs=3) as ppool, \
         tc.tile_pool(name="recip", bufs=3) as recpool:

        wo_sb = wpool2.tile([P, HL, D], BF)
        nc.sync.dma_start(out=wo_sb, in_=wo_l.rearrange("h p d -> p h d"))

        with tc.tile_pool(name="ps_att", bufs=1, space="PSUM") as ps_att, \
             tc.tile_pool(name="ps_acc", bufs=3, space="PSUM") as ps_acc:
            for qc in range(NQC):
                qsl = slice(qc * QCL, (qc + 1) * QCL)
                for h in range(HL):
                    av_ps = ps_acc.tile([P, QCL], F32, tag="av")
                    dn_ps = ps_acc.tile([P, QCL], F32, tag="dn")
                    for kp in range(NTT // 2):
                        # two k-tiles of scores share one exp activation
                        sc_ps = ps_att.tile([P, 2, QCL], F32, tag="scores")
                        for kl in range(2):
                            kt = kp * 2 + kl
                            nc.tensor.matmul(sc_ps[:, kl, :],
                                             ksT[:, h, ts(kt, P)],
                                             qsT[:, h, qsl],
                                             start=True, stop=True)
                        pT = ppool.tile([P, 2, QCL], BF, tag="pT")
                        nc.scalar.activation(out=pT, in_=sc_ps,
                                             func=mybir.ActivationFunctionType.Exp,
                                             scale=SCALE)
                        for kl in range(2):
                            kt = kp * 2 + kl
                            nc.tensor.matmul(av_ps, v_sb[:, h, kt, :],
                                             pT[:, kl, :],
                                             start=(kt == 0), stop=(kt == NTT - 1))
                            nc.tensor.matmul(dn_ps, ones128, pT[:, kl, :],
                                             start=(kt == 0), stop=(kt == NTT - 1))
                    rec = recpool.tile([P, QCL], F32, tag="rec")
                    nc.vector.reciprocal(out=rec, in_=dn_ps)
                    nc.vector.tensor_mul(outT[:, h, qsl], av_ps, rec)

        with tc.tile_pool(name="osb", bufs=4) as opool, \
             tc.tile_pool(name="ps_o", bufs=4, space="PSUM") as ps_o:
            for qt in range(t_len // P):
                for dc in range(D // 512):
                    o_ps = ps_o.tile([P, 512], F32, tag="o")
                    for h in range(HL):
                        nc.tensor.matmul(o_ps, outT[:, h, ts(qt, P)],
                                         wo_sb[:, h, ts(dc, 512)],
                                         start=(h == 0), stop=(h == HL - 1))
                    o_sb = opool.tile([P, 512], F32, tag="osb")
                    nc.vector.tensor_copy(out=o_sb, in_=o_ps)
                    nc.sync.dma_start(out=o_out[ts(qt, P), ts(dc, 512)], in_=o_sb)


_PROGRAM_CACHE = {}


def _build(t_len):
    if t_len in _PROGRAM_CACHE:
        return _PROGRAM_CACHE[t_len]
    nc = bacc.Bacc("TRN2", target_bir_lowering=False, debug=False,
                   num_devices=N_CORES)
    xT = nc.dram_tensor("xT", [D, t_len], BF, kind="ExternalInput").ap()
    wqa = nc.dram_tensor("wqa", [D, QR], BF, kind="ExternalInput").ap()
    wkva = nc.dram_tensor("wkva", [D, KVW], BF, kind="ExternalInput").ap()
    wqb = nc.dram_tensor("wqb", [QR, HL, DK], BF, kind="ExternalInput").ap()
    wkvbn = nc.dram_tensor("wkvbn", [KVR, HL, NOPE], BF, kind="ExternalInput").ap()
    wkvbv = nc.dram_tensor("wkvbv", [KVR, HL, VD], BF, kind="ExternalInput").ap()
    wo_l = nc.dram_tensor("wo_l", [HL, VD, D], BF, kind="ExternalInput").ap()
    cosA = nc.dram_tensor("cosA", [P, t_len], BF, kind="ExternalInput").ap()
    sinA2 = nc.dram_tensor("sinA2", [P, t_len], BF, kind="ExternalInput").ap()
    swapM = nc.dram_tensor("swapM", [P, P], BF, kind="ExternalInput").ap()
    o_out = nc.dram_tensor("o", [t_len, D], F32, kind="ExternalOutput").ap()
    aps = (xT, wqa, wkva, wqb, wkvbn, wkvbv, wo_l, cosA, sinA2, swapM, o_out)
    with tile.TileContext(nc) as tc, ExitStack() as ctx:
        _emit(nc, tc, ctx, t_len, aps)
    nc.compile()
    _PROGRAM_CACHE[t_len] = (nc, aps)
    return nc, aps


def make_in_maps(x, wq_a, q_a_norm_w, wq_b, wkv_a, kv_a_norm_w, wkv_b, wo,
                 t_len=T):
    """Host-side prep: fold norm weights, transpose x, build rope tables,
    shard weights per core."""
    x2 = np.asarray(x, np.float32).reshape(t_len, D)
    xT_np = np.ascontiguousarray(x2.T).astype(BF16)
    wqa_np = np.asarray(wq_a, np.float32).astype(BF16)
    # pad wkv_a: [c (512) | zeros (64) | k_pe (64)]
    wkva_f = np.asarray(wkv_a, np.float32)
    wkva_np = np.zeros((D, KVW), BF16)
    wkva_np[:, :KVR] = wkva_f[:, :KVR].astype(BF16)
    wkva_np[:, KVR + 64:] = wkva_f[:, KVR:].astype(BF16)
    # fold norm weights into b-projections
    wqb_f = (np.asarray(q_a_norm_w, np.float32)[:, None]
             * np.asarray(wq_b, np.float32)).reshape(QR, H, DK)
    wkvb_f = (np.asarray(kv_a_norm_w, np.float32)[:, None]
              * np.asarray(wkv_b, np.float32)).reshape(KVR, H, NOPE + VD)
    wo_f = np.asarray(wo, np.float32).reshape(H, VD, D)

    # rope tables, duplicated in both partition halves, sin sign-folded
    inv_freq = 1.0 / (10000.0 ** (np.arange(0, ROPE, 2, dtype=np.float32) / ROPE))
    tpos = np.arange(t_len, dtype=np.float32)
    freqs = np.outer(inv_freq, tpos)            # [HALF, T]
    cos = np.cos(freqs).astype(np.float32)
    sin = np.sin(freqs).astype(np.float32)
    cosA_np = np.concatenate([cos, cos, cos, cos], 0).astype(BF16)      # [128,T]
    sin2 = np.concatenate([-sin, sin], 0)                               # [64, T]
    sinA2_np = np.concatenate([sin2, sin2], 0).astype(BF16)
    # swapM: out rows 64:128 = half-swap of in rows 64:128 (wrt pe block)
    swap_np = np.zeros((P, P), np.float32)
    for i in range(HALF):
        swap_np[64 + HALF + i, 64 + i] = 1.0   # out[64+i] = in[96+i]
        swap_np[64 + i, 64 + HALF + i] = 1.0   # out[96+i] = in[64+i]
    swapM_np = swap_np.astype(BF16)

    shared = {
        "xT": xT_np, "wqa": wqa_np, "wkva": wkva_np,
        "cosA": cosA_np, "sinA2": sinA2_np, "swapM": swapM_np,
    }
    in_maps = []
    for i in range(N_CORES):
        hs = slice(i * HL, (i + 1) * HL)
        in_maps.append(dict(
            shared,
            wqb=np.ascontiguousarray(wqb_f[:, hs, :]).astype(BF16),
            wkvbn=np.ascontiguousarray(wkvb_f[:, hs, :NOPE]).astype(BF16),
            wkvbv=np.ascontiguousarray(wkvb_f[:, hs, NOPE:]).astype(BF16),
            wo_l=np.ascontiguousarray(wo_f[hs]).astype(BF16),
        ))
    return in_maps


def run_cores(inputs, t_len=T, trace=False, **kw):
    nc, _ = _build(t_len)
    in_maps = make_in_maps(**inputs, t_len=t_len)
    res = run_bass_kernel_spmd(nc, in_maps, list(range(N_CORES)), trace=trace, **kw)
    return res


def kernel(x, wq_a, q_a_norm_w, wq_b, wkv_a, kv_a_norm_w, wkv_b, wo):
    res = run_cores(dict(x=x, wq_a=wq_a, q_a_norm_w=q_a_norm_w, wq_b=wq_b,
                         wkv_a=wkv_a, kv_a_norm_w=kv_a_norm_w, wkv_b=wkv_b,
                         wo=wo))
    out = np.zeros((T, D), np.float32)
    for r in res.results:
        out += np.asarray(r["o"], np.float32)
    return out.reshape(1, T, D)


# revision 12
# speedup vs baseline: 1.7381x; 1.7381x over previous
"""DeepSeek MLA attention (B=1, T=2048, D=2048, H=16) on 8 trn2 NeuronCores.

Tensor-parallel over heads: 2 heads per core. wq_b / wkv_b output dims and
wo input dim are head-sharded; x, wq_a, wkv_a replicated. Each core produces
a partial [T, D] o_proj output; the host sums the 8 partials (the all-reduce).

All on-device activations live in transposed [feature-partition, T-free]
layouts so every matmul contraction sits on the partition axis. Matmul inputs
are bf16 (fp32 accumulation in PSUM). RMS-norm is a per-token column scale
(norm weights are folded into wq_b/wkv_b rows on the host); RoPE is applied
via a PE half-swap permutation matmul plus sign-folded sin/cos tables.
"""

import sys

if "/opt/trn_rl_repo" not in sys.path:
    sys.path.insert(0, "/opt/trn_rl_repo")

from contextlib import ExitStack

import ml_dtypes
import numpy as np

import concourse.bass as bass
import concourse.tile as tile
from concourse import bacc, mybir
from concourse.bass import ts
from concourse.bass_utils import run_bass_kernel_spmd
from concourse.masks import make_identity

BF16 = ml_dtypes.bfloat16
F32 = mybir.dt.float32
BF = mybir.dt.bfloat16

T, D = 2048, 2048
H, DK = 16, 128
ROPE, NOPE, VD = 64, 64, 128
HALF = ROPE // 2
KVR, QR = 512, 768
KVW = KVR + 128  # wkv_a padded: [0:512]=c, [512:576]=0, [576:640]=k_pe
SCALE = DK ** -0.5
EPS = float(np.finfo(np.float32).eps)
N_CORES = 8
HL = H // N_CORES  # heads per core

TCH = 512                 # T-chunk for projection phase
QC = 512                  # q-chunk for attention phase
P = 128


def _emit(nc, tc, ctx, t_len, aps):
    (xT, wqa, wkva, wqb, wkvbn, wkvbv, wo_l, cosA, sinA2, swapM, o_out) = aps
    NT = t_len // TCH          # projection chunks
    NTT = t_len // P           # token tiles
    QCL = min(QC, t_len)       # attention q-chunk size
    NQC = t_len // QCL         # attention q chunks
    TPC = TCH // P             # token tiles per chunk
    QRT = QR // P   # 6
    CRT = KVR // P  # 4
    DTN = D // P    # 16

    consts = ctx.enter_context(tc.tile_pool(name="consts", bufs=1))
    persist = ctx.enter_context(tc.tile_pool(name="persist", bufs=1))

    # constants
    ones128 = consts.tile([P, P], BF)
    nc.gpsimd.memset(ones128, 1.0)
    ident = consts.tile([P, P], F32)
    make_identity(nc, ident)
    eps_ap = consts.tile([P, 1], F32)
    nc.vector.memset(eps_ap, EPS)

    # persistent activations
    qsT = persist.tile([P, HL, t_len], BF)       # [DK, h, T] (nope | roped pe)
    ksT = persist.tile([P, HL, t_len], BF)
    v_sb = persist.tile([P, HL, NTT, VD], BF)    # [token-in-tile, h, ttile, VD]
    outT = persist.tile([P, HL, t_len], BF)      # [VD, h, T]
    s_cT = persist.tile([P, NTT], F32)           # per-token kv rms scale, col layout

    # ================= phase 1: projections =================
    with tc.tile_pool(name="w1", bufs=1) as wpool, \
         tc.tile_pool(name="xchunk", bufs=2) as xpool, \
         tc.tile_pool(name="raw", bufs=2) as rawpool, \
         tc.tile_pool(name="sq", bufs=3) as sqpool, \
         tc.tile_pool(name="scales", bufs=2) as scpool, \
         tc.tile_pool(name="ropetmp", bufs=2) as tmpool, \
         tc.tile_pool(name="ps_proj", bufs=3, space="PSUM") as ps_proj, \
         tc.tile_pool(name="ps_sum", bufs=1, space="PSUM") as ps_sum, \
         tc.tile_pool(name="ps_small", bufs=3, space="PSUM") as ps_small:

        # interleave chunk-0 x tiles with the projection weights so the
        # first matmul group can start after ~3 small DMAs
        wqa_sb = wpool.tile([P, DTN, QR], BF)
        wkva_sb = wpool.tile([P, DTN, KVW], BF)
        xc0 = xpool.tile([P, DTN, TCH], BF, tag="xc")
        for dt in range(DTN):
            nc.sync.dma_start(out=xc0[:, dt, :], in_=xT[ts(dt, P), 0:TCH])
            nc.sync.dma_start(out=wqa_sb[:, dt, :], in_=wqa[ts(dt, P), :])
            nc.sync.dma_start(out=wkva_sb[:, dt, :], in_=wkva[ts(dt, P), :])
        wqb_sb = wpool.tile([P, QRT, HL, DK], BF)
        nc.sync.dma_start(out=wqb_sb, in_=wqb.rearrange("(r p) h d -> p r h d", p=P))
        wkvbn_sb = wpool.tile([P, CRT, HL, NOPE], BF)
        nc.sync.dma_start(out=wkvbn_sb, in_=wkvbn.rearrange("(r p) h d -> p r h d", p=P))
        wkvbv_sb = wpool.tile([P, CRT, HL, VD], BF)
        nc.sync.dma_start(out=wkvbv_sb, in_=wkvbv.rearrange("(r p) h d -> p r h d", p=P))
        cosA_sb = wpool.tile([P, t_len], BF)
        nc.sync.dma_start(out=cosA_sb, in_=cosA)
        sinA2_sb = wpool.tile([P, t_len], BF)
        nc.sync.dma_start(out=sinA2_sb, in_=sinA2)
        swapM_sb = wpool.tile([P, P], BF)
        nc.sync.dma_start(out=swapM_sb, in_=swapM)

        for c in range(NT):
            tsl = slice(c * TCH, (c + 1) * TCH)
            if c == 0:
                xc = xc0
            else:
                xc = xpool.tile([P, DTN, TCH], BF, tag="xc")
                nc.sync.dma_start(
                    out=xc, in_=xT[:, tsl].rearrange("(dt p) t -> p dt t", p=P)
                )

            qa_r = rawpool.tile([P, QRT, TCH], BF, tag="qa_raw")
            c_r = rawpool.tile([P, CRT, TCH], BF, tag="c_raw")
            kpe_r = rawpool.tile([P, TCH], BF, tag="kpe_raw")  # rows 64:128 = k_pe

            sq_q_ps = ps_sum.tile([P, TCH], F32, tag="sq_q")
            sq_c_ps = ps_sum.tile([P, TCH], F32, tag="sq_c")

            scale_q = scpool.tile([P, TCH], F32, tag="scale_q")
            scale_c = scpool.tile([P, TCH], F32, tag="scale_c")

            # x @ [wq_a | wkv_a] (transposed). The ones-matmul (sum of
            # squares) for group r is emitted after group r+1's matmuls so
            # the PE never waits on the ACT Square latency.
            pending = None

            def flush_pending():
                nonlocal pending
                if pending is not None:
                    tgt, sqt, st, sp = pending
                    nc.tensor.matmul(tgt, ones128, sqt, start=st, stop=sp)
                    pending = None

            for r in range(QRT + CRT + 1):
                ps = ps_proj.tile([P, TCH], F32, tag="proj")
                if r < QRT:
                    w, col = wqa_sb, ts(r, P)
                elif r < QRT + CRT:
                    w, col = wkva_sb, ts(r - QRT, P)
                else:
                    w, col = wkva_sb, ts(CRT, P)  # padded pe block -> rows 64:128
                for dt in range(DTN):
                    nc.tensor.matmul(
                        ps, w[:, dt, col], xc[:, dt, :],
                        start=(dt == 0), stop=(dt == DTN - 1),
                    )
                flush_pending()
                if r < QRT:
                    nc.vector.tensor_copy(out=qa_r[:, r, :], in_=ps)
                    sq = sqpool.tile([P, TCH], BF, tag="sq")
                    nc.scalar.activation(out=sq, in_=ps,
                                         func=mybir.ActivationFunctionType.Square)
                    pending = (sq_q_ps, sq, r == 0, r == QRT - 1)
                elif r < QRT + CRT:
                    rc = r - QRT
                    nc.vector.tensor_copy(out=c_r[:, rc, :], in_=ps)
                    sq = sqpool.tile([P, TCH], BF, tag="sq")
                    nc.scalar.activation(out=sq, in_=ps,
                                         func=mybir.ActivationFunctionType.Square)
                    pending = (sq_c_ps, sq, rc == 0, rc == CRT - 1)
                else:
                    nc.vector.tensor_copy(out=kpe_r, in_=ps)
                # rms scale chains, emitted as soon as their sumsq closes
                if r == QRT:  # q sumsq flushed above
                    tmp_q = scpool.tile([P, TCH], F32, tag="scale_tmp")
                    nc.scalar.activation(out=tmp_q, in_=sq_q_ps,
                                         func=mybir.ActivationFunctionType.Sqrt,
                                         scale=1.0 / QR, bias=eps_ap)
                    nc.vector.reciprocal(out=scale_q, in_=tmp_q)
            flush_pending()  # c sumsq (last group has no successor)
            tmp_c = scpool.tile([P, TCH], F32, tag="scale_tmp")
            nc.scalar.activation(out=tmp_c, in_=sq_c_ps,
                                 func=mybir.ActivationFunctionType.Sqrt,
                                 scale=1.0 / KVR, bias=eps_ap)
            nc.vector.reciprocal(out=scale_c, in_=tmp_c)

            # rope tables pre-scaled by the q rms scale (rows 64:128 only)
            cos_s = scpool.tile([P, TCH], BF, tag="cos_s")
            sin_s = scpool.tile([P, TCH], BF, tag="sin_s")
            nc.vector.tensor_mul(cos_s[64:128, :], cosA_sb[64:128, tsl],
                                 scale_q[64:128, :])
            nc.vector.tensor_mul(sin_s[64:128, :], sinA2_sb[64:128, tsl],
                                 scale_q[64:128, :])

            # ---- b-projections. PE order: qsT/kn matmul groups first, then
            # the scale_c transposes, v groups, and finally the rope swaps —
            # so every PE instruction's inputs are ready when it issues.
            q_ps = []
            for h in range(HL):
                ps = ps_proj.tile([P, TCH], F32, tag="proj")
                for r in range(QRT):
                    nc.tensor.matmul(ps, wqb_sb[:, r, h, :], qa_r[:, r, :],
                                     start=(r == 0), stop=(r == QRT - 1))
                nc.vector.tensor_mul(qsT[0:64, h, tsl], ps[0:64, :],
                                     scale_q[0:64, :])
                qpe_raw = tmpool.tile([P, TCH], BF, tag=f"qpe_raw{h}")
                nc.vector.tensor_copy(out=qpe_raw, in_=ps)
                q_ps.append(qpe_raw)

            kn_ps_l = []
            for h in range(HL):
                kn_ps = ps_small.tile([64, TCH], F32, tag="small")
                for r in range(CRT):
                    nc.tensor.matmul(kn_ps, wkvbn_sb[:, r, h, :], c_r[:, r, :],
                                     start=(r == 0), stop=(r == CRT - 1))
                nc.vector.tensor_mul(ksT[0:64, h, tsl], kn_ps, scale_c[0:64, :])
                kn_ps_l.append(kn_ps)

            # column-layout copy of scale_c for per-partition v scaling
            for j in range(TPC):
                tr_ps = ps_small.tile([P, P], F32, tag="small")
                nc.tensor.transpose(tr_ps, scale_c[:, ts(j, P)], ident)
                nc.vector.tensor_copy(out=s_cT[:, c * TPC + j: c * TPC + j + 1],
                                      in_=tr_ps[:, 0:1])

            for h in range(HL):
                for j in range(TPC):
                    tt = c * TPC + j
                    v_ps = ps_small.tile([P, VD], F32, tag="small")
                    for r in range(CRT):
                        nc.tensor.matmul(v_ps, c_r[:, r, ts(j, P)],
                                         wkvbv_sb[:, r, h, :],
                                         start=(r == 0), stop=(r == CRT - 1))
                    nc.vector.tensor_scalar_mul(v_sb[:, h, tt, :], v_ps,
                                                s_cT[:, tt:tt + 1])

            # rope swaps (inputs were evicted long before these issue)
            for h in range(HL):
                sw_ps = ps_proj.tile([P, TCH], F32, tag="proj")
                nc.tensor.matmul(sw_ps, swapM_sb, q_ps[h], start=True, stop=True)
                m1 = tmpool.tile([P, TCH], BF, tag="rope_m1")
                m2 = tmpool.tile([P, TCH], BF, tag="rope_m2")
                nc.vector.tensor_mul(m1[64:128, :], q_ps[h][64:128, :],
                                     cos_s[64:128, :])
                nc.vector.tensor_mul(m2[64:128, :], sw_ps[64:128, :],
                                     sin_s[64:128, :])
                nc.vector.tensor_add(qsT[64:128, h, tsl], m1[64:128, :],
                                     m2[64:128, :])

            ksw_ps = ps_proj.tile([P, TCH], F32, tag="proj")
            nc.tensor.matmul(ksw_ps, swapM_sb, kpe_r, start=True, stop=True)
            km1 = tmpool.tile([P, TCH], BF, tag="rope_m1")
            km2 = tmpool.tile([P, TCH], BF, tag="rope_m2")
            nc.vector.tensor_mul(km1[64:128, :], kpe_r[64:128, :],
                                 cosA_sb[64:128, tsl])
            nc.vector.tensor_mul(km2[64:128, :], ksw_ps[64:128, :],
                                 sinA2_sb[64:128, tsl])
            nc.vector.tensor_add(ksT[64:128, 0, tsl], km1[64:128, :],
                                 km2[64:128, :])
            nc.vector.tensor_add(ksT[64:128, 1, tsl], km1[64:128, :],
                                 km2[64:128, :])

    # ================= phase 2+3: attention, then o_proj =================
    with tc.tile_pool(name="w2", bufs=1) as wpool2, \
         tc.tile_pool(name="probs", bufs=4) as ppool, \
         tc.tile_pool(name="recip", bufs=3) as recpool:

        wo_sb = wpool2.tile([P, HL, D], BF)
        nc.sync.dma_start(out=wo_sb, in_=wo_l.rearrange("h p d -> p h d"))

        with tc.tile_pool(name="ps_att", bufs=2, space="PSUM") as ps_att, \
             tc.tile_pool(name="ps_acc", bufs=2, space="PSUM") as ps_acc:
            # software-pipelined: each pair's av/dn matmuls are emitted after
            # the NEXT pair's score matmuls, so the PE (strict FIFO) never
            # waits on the exp latency of the pair it just produced.
            pend = None   # (pT, base kt, av_ps, dn_ps, unit)

            def flush_av(nc):
                nonlocal pend
                if pend is None:
                    return
                pT, kb, av_p, dn_p, unit = pend
                for kl in range(2):
                    kt = kb + kl
                    nc.tensor.matmul(av_p, v_sb[:, unit[1], kt, :], pT[:, kl, :],
                                     start=(kt == 0), stop=(kt == NTT - 1))
                    nc.tensor.matmul(dn_p, ones128, pT[:, kl, :],
                                     start=(kt == 0), stop=(kt == NTT - 1))
                pend = None

            finals = []
            for qc in range(NQC):
                qsl = slice(qc * QCL, (qc + 1) * QCL)
                for h in range(HL):
                    av_ps = ps_acc.tile([P, QCL], F32, tag="av")
                    dn_ps = ps_acc.tile([P, QCL], F32, tag="dn")
                    for kp in range(NTT // 2):
                        sc_ps = ps_att.tile([P, 2, QCL], F32, tag="scores")
                        for kl in range(2):
                            kt = kp * 2 + kl
                            nc.tensor.matmul(sc_ps[:, kl, :],
                                             ksT[:, h, ts(kt, P)],
                                             qsT[:, h, qsl],
                                             start=True, stop=True)
                        pT = ppool.tile([P, 2, QCL], BF, tag="pT")
                        nc.scalar.activation(out=pT, in_=sc_ps,
                                             func=mybir.ActivationFunctionType.Exp,
                                             scale=SCALE)
                        flush_av(nc)
                        pend = (pT, kp * 2, av_ps, dn_ps, (qc, h))
                        # previous unit's epilogue, after this unit started
                        while finals:
                            f_av, f_dn, f_qsl, f_h = finals.pop()
                            rec = recpool.tile([P, QCL], F32, tag="rec")
                            nc.vector.reciprocal(out=rec, in_=f_dn)
                            nc.vector.tensor_mul(outT[:, f_h, f_qsl], f_av, rec)
                    finals.append((av_ps, dn_ps, qsl, h))
            flush_av(nc)
            for f_av, f_dn, f_qsl, f_h in finals:
                rec = recpool.tile([P, QCL], F32, tag="rec")
                nc.vector.reciprocal(out=rec, in_=f_dn)
                nc.vector.tensor_mul(outT[:, f_h, f_qsl], f_av, rec)

        with tc.tile_pool(name="osb", bufs=3) as opool, \
             tc.tile_pool(name="ps_o", bufs=2, space="PSUM") as ps_o:
            for qt in range(t_len // P):
                o_ps = ps_o.tile([P, D // 512, 512], F32, tag="o")
                for h in range(HL):
                    for dc in range(D // 512):
                        nc.tensor.matmul(o_ps[:, dc, :], outT[:, h, ts(qt, P)],
                                         wo_sb[:, h, ts(dc, 512)],
                                         start=(h == 0), stop=(h == HL - 1))
                o_sb = opool.tile([P, D], F32, tag="osb")
                nc.vector.tensor_copy(out=o_sb, in_=o_ps)
                nc.sync.dma_start(out=o_out[ts(qt, P), :], in_=o_sb)


_PROGRAM_CACHE = {}


def _build(t_len):
    if t_len in _PROGRAM_CACHE:
        return _PROGRAM_CACHE[t_len]
    nc = bacc.Bacc("TRN2", target_bir_lowering=False, debug=False,
                   num_devices=N_CORES)
    xT = nc.dram_tensor("xT", [D, t_len], BF, kind="ExternalInput").ap()
    wqa = nc.dram_tensor("wqa", [D, QR], BF, kind="ExternalInput").ap()
    wkva = nc.dram_tensor("wkva", [D, KVW], BF, kind="ExternalInput").ap()
    wqb = nc.dram_tensor("wqb", [QR, HL, DK], BF, kind="ExternalInput").ap()
    wkvbn = nc.dram_tensor("wkvbn", [KVR, HL, NOPE], BF, kind="ExternalInput").ap()
    wkvbv = nc.dram_tensor("wkvbv", [KVR, HL, VD], BF, kind="ExternalInput").ap()
    wo_l = nc.dram_tensor("wo_l", [HL, VD, D], BF, kind="ExternalInput").ap()
    cosA = nc.dram_tensor("cosA", [P, t_len], BF, kind="ExternalInput").ap()
    sinA2 = nc.dram_tensor("sinA2", [P, t_len], BF, kind="ExternalInput").ap()
    swapM = nc.dram_tensor("swapM", [P, P], BF, kind="ExternalInput").ap()
    o_out = nc.dram_tensor("o", [t_len, D], F32, kind="ExternalOutput").ap()
    aps = (xT, wqa, wkva, wqb, wkvbn, wkvbv, wo_l, cosA, sinA2, swapM, o_out)
    with tile.TileContext(nc) as tc, ExitStack() as ctx:
        _emit(nc, tc, ctx, t_len, aps)
    nc.compile()
    _PROGRAM_CACHE[t_len] = (nc, aps)
    return nc, aps


def make_in_maps(x, wq_a, q_a_norm_w, wq_b, wkv_a, kv_a_norm_w, wkv_b, wo,
                 t_len=T):
    """Host-side prep: fold norm weights, transpose x, build rope tables,
    shard weights per core."""
    x2 = np.asarray(x, np.float32).reshape(t_len, D)
    xT_np = np.ascontiguousarray(x2.T).astype(BF16)
    wqa_np = np.asarray(wq_a, np.float32).astype(BF16)
    # pad wkv_a: [c (512) | zeros (64) | k_pe (64)]
    wkva_f = np.asarray(wkv_a, np.float32)
    wkva_np = np.zeros((D, KVW), BF16)
    wkva_np[:, :KVR] = wkva_f[:, :KVR].astype(BF16)
    wkva_np[:, KVR + 64:] = wkva_f[:, KVR:].astype(BF16)
    # fold norm weights into b-projections
    wqb_f = (np.asarray(q_a_norm_w, np.float32)[:, None]
             * np.asarray(wq_b, np.float32)).reshape(QR, H, DK)
    wkvb_f = (np.asarray(kv_a_norm_w, np.float32)[:, None]
              * np.asarray(wkv_b, np.float32)).reshape(KVR, H, NOPE + VD)
    wo_f = np.asarray(wo, np.float32).reshape(H, VD, D)

    # rope tables, duplicated in both partition halves, sin sign-folded
    inv_freq = 1.0 / (10000.0 ** (np.arange(0, ROPE, 2, dtype=np.float32) / ROPE))
    tpos = np.arange(t_len, dtype=np.float32)
    freqs = np.outer(inv_freq, tpos)            # [HALF, T]
    cos = np.cos(freqs).astype(np.float32)
    sin = np.sin(freqs).astype(np.float32)
    cosA_np = np.concatenate([cos, cos, cos, cos], 0).astype(BF16)      # [128,T]
    sin2 = np.concatenate([-sin, sin], 0)                               # [64, T]
    sinA2_np = np.concatenate([sin2, sin2], 0).astype(BF16)
    # swapM: out rows 64:128 = half-swap of in rows 64:128 (wrt pe block)
    swap_np = np.zeros((P, P), np.float32)
    for i in range(HALF):
        swap_np[64 + HALF + i, 64 + i] = 1.0   # out[64+i] = in[96+i]
        swap_np[64 + i, 64 + HALF + i] = 1.0   # out[96+i] = in[64+i]
    swapM_np = swap_np.astype(BF16)

    shared = {
        "xT": xT_np, "wqa": wqa_np, "wkva": wkva_np,
        "cosA": cosA_np, "sinA2": sinA2_np, "swapM": swapM_np,
    }
    in_maps = []
    for i in range(N_CORES):
        hs = slice(i * HL, (i + 1) * HL)
        in_maps.append(dict(
            shared,
            wqb=np.ascontiguousarray(wqb_f[:, hs, :]).astype(BF16),
            wkvbn=np.ascontiguousarray(wkvb_f[:, hs, :NOPE]).astype(BF16),
            wkvbv=np.ascontiguousarray(wkvb_f[:, hs, NOPE:]).astype(BF16),
            wo_l=np.ascontiguousarray(wo_f[hs]).astype(BF16),
        ))
    return in_maps


def run_cores(inputs, t_len=T, trace=False, **kw):
    nc, _ = _build(t_len)
    in_maps = make_in_maps(**inputs, t_len=t_len)
    res = run_bass_kernel_spmd(nc, in_maps, list(range(N_CORES)), trace=trace, **kw)
    return res


def kernel(x, wq_a, q_a_norm_w, wq_b, wkv_a, kv_a_norm_w, wkv_b, wo):
    res = run_cores(dict(x=x, wq_a=wq_a, q_a_norm_w=q_a_norm_w, wq_b=wq_b,
                         wkv_a=wkv_a, kv_a_norm_w=kv_a_norm_w, wkv_b=wkv_b,
                         wo=wo))
    out = np.zeros((T, D), np.float32)
    for r in res.results:
        out += np.asarray(r["o"], np.float32)
    return out.reshape(1, T, D)
